# revision 28
# baseline (speedup 1.0000x reference)
"""Multi-head attention forward, sharded head-parallel across 8 NeuronCores.

Per core c (heads 2c, 2c+1):
  qT/kT/vT = (x @ W{q,k,v}_c.T).T        computed as W.T-tiled matmuls vs xT
  scoresT  = kT_chunk.T @ qT             [k-pos partitions, q-pos free]
  probsT   = exp(scoresT), causal handled by suffix windows + DVE tri-mul
  av+rowsum: out.T = [v | 1].T @ probsT  (ones column yields softmax denom)
  normalize by PE-broadcast reciprocal, then out_projT partial (bf16)
Host: sum the 8 partial [1024, 4096] outputs, transpose, add bias.
"""
import sys
from contextlib import ExitStack

sys.path.insert(0, "/opt/trn_rl_repo")

import ml_dtypes
import numpy as np

BF16 = ml_dtypes.bfloat16

B, S, D = 2, 2048, 1024
H, HD = 16, 64
NCORES = 8
SEC = 128           # output dims per core per section (2 heads * 64)
BS = B * S          # 4096
NT = BS // 512      # 8 seq tiles of 512
EC = D // 128       # 8 embed chunks
QT = S // 512       # 4 q-tiles per (b,h)
KC = S // 128       # 16 k-chunks per (b,h)

_cache = {}


def _build(mask_mode):
    import concourse.bass as bass
    import concourse.tile as tile
    from concourse import bacc, mybir

    f32 = mybir.dt.float32
    bf16 = mybir.dt.bfloat16
    Exp = mybir.ActivationFunctionType.Exp

    nc = bacc.Bacc("TRN2", target_bir_lowering=False, debug=False,
                   num_devices=NCORES)

    # xh: host-blocked x layout [128, n, ec, 512] so each per-n DMA reads
    # 8KB-contiguous runs per partition
    xh = nc.dram_tensor("xh", [128, NT * EC * 512], bf16,
                        kind="ExternalInput")
    # wqh: host-blocked weights [128, ec, 3*SEC] -> 6KB runs per partition
    wqh = nc.dram_tensor("wqh", [128, EC * 3 * SEC], bf16,
                         kind="ExternalInput")
    woT = nc.dram_tensor("woT", [SEC, D], bf16, kind="ExternalInput")
    # consts: [:, 0:128] = eye(128); [:, 128:256] = lower-tri (p <= c)
    consts = nc.dram_tensor("consts", [128, 256], bf16, kind="ExternalInput")
    if mask_mode == "general":
        maskT = nc.dram_tensor("maskT", [S, S], bf16, kind="ExternalInput")
    out_pT = nc.dram_tensor("out_pT", [D, BS], bf16, kind="ExternalOutput")

    with tile.TileContext(nc) as tc:
        with (
            nc.allow_low_precision(reason="bf16 transpose/accum passthrough"),
            tc.tile_pool(name="singles", bufs=1) as singles,
            tc.tile_pool(name="qkv", bufs=1) as qkv,
            tc.tile_pool(name="xp", bufs=4) as xp,
            tc.tile_pool(name="v1p", bufs=1) as v1p,
            tc.tile_pool(name="pp", bufs=4) as pp,
            tc.tile_pool(name="np_", bufs=2) as np_,
            tc.tile_pool(name="fo", bufs=2) as fo,
        ):
            # ---- ident + qkv weights head the sync (hardware DMA) queue;
            # the gpsimd software-DGE queue is slow to start, so only the
            # late-needed woT/tri ride it ----
            ident = singles.tile([128, 128], bf16)
            nc.sync.dma_start(out=ident[:], in_=consts[:, 0:128])
            w_sb = singles.tile([128, EC, 3 * SEC], bf16)
            nc.sync.dma_start(
                out=w_sb[:],
                in_=wqh.rearrange("p (ec c) -> p ec c", ec=EC)[:])
            woT_sb = singles.tile([128, D], bf16)
            nc.gpsimd.dma_start(out=woT_sb[:], in_=woT[:])
            tri = singles.tile([128, 128], bf16)
            nc.gpsimd.dma_start(out=tri[:], in_=consts[:, 128:256])
            ones1 = singles.tile([1, 64], f32)
            nc.vector.memset(ones1[:], 1.0)

            # ---- x load: per-n tiles, n-major; 8KB runs per partition ----
            xhr = xh.rearrange("p (n ec s) -> p n ec s", n=NT, ec=EC)
            xns = []
            for n in range(NT):
                xn = qkv.tile([128, EC, 512], bf16, name=f"xn{n}")
                nc.sync.dma_start(out=xn[:], in_=xhr[:, n, :, :])
                xns.append(xn)
            qT = qkv.tile([128, BS], bf16)
            kT = qkv.tile([128, BS], bf16)
            vT = qkv.tile([128, BS], bf16)
            ocat = qkv.tile([128, BS], bf16)

            # v1[b]: [128 kpos, chunk, head, 64 v-dims + ones col]
            v1s = []
            for b in range(B):
                v1 = v1p.tile([128, KC, 2, HD + 1], bf16, name=f"v1{b}")
                nc.vector.memset(v1[:, :, :, HD], 1.0)
                v1s.append(v1)

            ctx = ExitStack()
            # scores/pos pools open up front so attention never waits on a
            # pool handoff; psQ is the slim QKV accumulator ring
            psS = ctx.enter_context(
                tc.tile_pool(name="psS", bufs=2, space="PSUM"))
            psO = ctx.enter_context(
                tc.tile_pool(name="psO", bufs=2, space="PSUM"))
            ctxQ = ExitStack()
            psQ = ctxQ.enter_context(
                tc.tile_pool(name="psQ", bufs=2, space="PSUM"))
            psF_box = [None]

            # PE warm-up: ~4us of dummy matmuls on ident while x loads, so
            # the HAM clock gate is already 8/8 when stage A starts
            warm = psS.tile([128, 128], f32, tag="ps", name="warm")
            for _ in range(75):
                nc.tensor.matmul(warm[:], ident[:], ident[:],
                                 start=True, stop=True)

            # transpose one [128 vdims, 128 seq] chunk -> v1[b][:, i, :, 0:64]
            def emit_transpose(b, i):
                base = S * b
                pt = psS.tile([128, 128], bf16, tag="ps", name="pt")
                nc.tensor.transpose(
                    pt[:], vT[:, base + 128 * i:base + 128 * (i + 1)],
                    ident[:])
                ptv = pt.rearrange("p (lh c) -> p lh c", lh=2)
                nc.vector.tensor_copy(v1s[b][:, i, :, 0:HD], ptv[:])

            # one q/k/v projection slab: 8 accumulating MMs + one cast,
            # emitted in two halves so filler stays fine-grained
            pa_box = {}

            def emit_proj_half(n, kind, dstT, half, scalar_cast):
                if half == 0:
                    pa_box[(n, kind)] = psQ.tile([128, 512], f32, tag="pa",
                                                 name=f"pa{n}_{kind}")
                pa = pa_box[(n, kind)]
                for ec in range(4 * half, 4 * half + 4):
                    nc.tensor.matmul(
                        pa[:], w_sb[:, ec, 128 * kind:128 * (kind + 1)],
                        xns[n][:, ec, :], start=(ec == 0), stop=(ec == EC - 1))
                if half == 1:
                    sl = slice(512 * n, 512 * (n + 1))
                    if scalar_cast:
                        nc.scalar.copy(dstT[:, sl], pa[:])
                    else:
                        nc.vector.tensor_copy(dstT[:, sl], pa[:])
                    del pa_box[(n, kind)]

            def emit_proj(n, kind, dstT, scalar_cast):
                emit_proj_half(n, kind, dstT, 0, scalar_cast)
                emit_proj_half(n, kind, dstT, 1, scalar_cast)

            # ---- stage A part 1: n0-3 (batch 0) + its transposes ----
            for n in range(4):
                for kind, dstT in ((0, qT), (1, kT), (2, vT)):
                    emit_proj(n, kind, dstT, scalar_cast=True)
                for i in range(4 * (n - 1) if n else 0, 4 * n):
                    emit_transpose(0, i)
            for i in range(12, 16):
                emit_transpose(0, i)

            # filler thunks: QKV n4-7 + b1 transposes, fed into b0's
            # attention stream to fill PE gaps (in-order PE queue)
            filler = []
            for n in range(4, NT):
                for kind, dstT in ((0, qT), (1, kT), (2, vT)):
                    filler.append((emit_proj_half, (n, kind, dstT, 0, False)))
                    filler.append((emit_proj_half, (n, kind, dstT, 1, False)))
                for i in range(4 * (n - 4), 4 * (n - 3)):
                    filler.append((emit_transpose, (1, i)))
            fill_pos = [0]

            def drain_filler(k):
                while fill_pos[0] < len(filler) and k > 0:
                    fn, args = filler[fill_pos[0]]
                    fn(*args)
                    fill_pos[0] += 1
                    k -= 1

            def emit_av(v1, pos, pend, njc):
                ji, j, off, pr = pend
                prv = pr.rearrange("p (lh q) -> p lh q", lh=2)
                for lh in range(2):
                    nc.tensor.matmul(pos[lh][:, off:512],
                                     v1[:, j, lh, :],
                                     prv[:, lh, off:512],
                                     start=(ji == 0), stop=(ji == njc - 1))

            def emit_outproj(n, use_scalar):
                ssl = slice(512 * n, 512 * (n + 1))
                ft = fo.tile([128, EC, 512], bf16, tag="ft")
                for oc in range(EC):
                    osl = slice(128 * oc, 128 * (oc + 1))
                    pf = psF_box[0].tile([128, 512], f32, tag="pf")
                    nc.tensor.matmul(pf[:], woT_sb[:, osl],
                                     ocat[:, ssl],
                                     start=True, stop=True)
                    if use_scalar:
                        nc.scalar.copy(ft[:, oc, :], pf[:])
                    else:
                        nc.vector.tensor_copy(ft[:, oc, :], pf[:])
                out_r = out_pT.rearrange("(oc p) s -> p oc s", p=128)
                nc.sync.dma_start(out=out_r[:, :, ssl], in_=ft[:])

            def emit_norm(b, t, lh, ol, rc):
                base = S * b
                hsl = slice(64 * lh, 64 * (lh + 1))
                qsl = slice(base + 512 * t, base + 512 * (t + 1))
                pb = psO.tile([HD + 1, 512], f32, tag="po", name="pb")
                nc.tensor.matmul(pb[0:HD, :], ones1[:], rc[:],
                                 start=True, stop=True)
                bs_ = np_.tile([64, 512], bf16, tag="bs")
                nc.vector.tensor_copy(bs_[:], pb[0:HD, :])
                nc.gpsimd.tensor_mul(ocat[hsl, qsl], ol[:], bs_[:])

            # b1-era fill queue: norm + out_proj units, drained a few per
            # chunk so the PE stream stays dense through the diag-chunk eras
            cfill = []
            cpos = [0]
            in_loop = [True]
            ft_box = {}
            out_r = out_pT.rearrange("(oc p) s -> p oc s", p=128)

            def emit_oc(n, oc):
                if oc == 0:
                    ft_box[n] = fo.tile([128, EC, 512], bf16, tag="ft",
                                        name=f"ft{n}")
                ft = ft_box[n]
                ssl = slice(512 * n, 512 * (n + 1))
                pf = psF_box[0].tile([128, 512], f32, tag="pf")
                nc.tensor.matmul(pf[:], woT_sb[:, 128 * oc:128 * (oc + 1)],
                                 ocat[:, ssl], start=True, stop=True)
                if oc % 2 == 1:
                    nc.scalar.copy(ft[:, oc, :], pf[:])
                else:
                    nc.vector.tensor_copy(ft[:, oc, :], pf[:])
                if oc % 2 == 1:
                    ssl = slice(512 * n, 512 * (n + 1))
                    nc.sync.dma_start(out=out_r[:, oc - 1:oc + 1, ssl],
                                      in_=ft[:, oc - 1:oc + 1, :])
                    if oc == EC - 1:
                        ft_box.pop(n)

            def push_outproj(n):
                for oc in range(EC):
                    cfill.append((emit_oc, (n, oc)))

            def drain_cfill(k):
                while cpos[0] < len(cfill) and k > 0:
                    fn, args = cfill[cpos[0]]
                    fn(*args)
                    cpos[0] += 1
                    k -= 1

            # ---- stage B (+ C interleaved into b1) ----
            for b in range(B):
                base = S * b
                v1 = v1s[b]
                norm_tiles = []
                if b == 1:
                    drain_cfill(6)
                for t in range(QT):
                    njc = 4 * t + 4 if mask_mode == "causal" else KC
                    pos = [psO.tile([HD + 1, 512], f32, tag="po",
                                    name=f"po{_lh}")
                           for _lh in range(2)]
                    pend = None
                    for ji, j in enumerate(range(njc)):
                        diag = mask_mode == "causal" and j >= 4 * t
                        off = 128 * (j - 4 * t) if diag else 0
                        ksl = slice(base + 128 * j, base + 128 * (j + 1))
                        ps = psS.tile([128, 1024], f32, tag="ps")
                        for lh in range(2):
                            hsl = slice(64 * lh, 64 * (lh + 1))
                            qsl = slice(base + 512 * t + off,
                                        base + 512 * (t + 1))
                            nc.tensor.matmul(
                                ps[:, 512 * lh + off:512 * (lh + 1)],
                                kT[hsl, ksl], qT[hsl, qsl],
                                start=True, stop=True)
                        pr = pp.tile([128, 1024], bf16, tag="pr")
                        psv = ps.rearrange("p (lh q) -> p lh q", lh=2)
                        prv = pr.rearrange("p (lh q) -> p lh q", lh=2)
                        nc.scalar.activation(prv[:, :, off:512],
                                             psv[:, :, off:512], Exp)
                        if diag:
                            for lh in range(2):
                                sub = pr[:, 512 * lh + off:
                                         512 * lh + off + 128]
                                nc.vector.tensor_mul(sub, sub, tri[:])
                        elif mask_mode == "general":
                            msk = xp.tile([128, 512], bf16, tag="msk")
                            nc.sync.dma_start(
                                out=msk[:],
                                in_=maskT[128 * j:128 * (j + 1),
                                          512 * t:512 * (t + 1)])
                            for lh in range(2):
                                sub = pr[:, 512 * lh:512 * (lh + 1)]
                                nc.vector.tensor_mul(sub, sub, msk[:])
                        if pend is not None:
                            emit_av(v1, pos, pend, njc)
                        pend = (ji, j, off, pr)
                        if b == 0:
                            drain_filler(1)
                        else:
                            drain_cfill(2 if t < 2 else 3)
                    emit_av(v1, pos, pend, njc)
                    for lh in range(2):
                        ol = np_.tile([HD, 512], f32, tag="ol", bufs=8,
                                      name=f"ol{t}{lh}")
                        nc.vector.tensor_copy(ol[:], pos[lh][0:HD, :])
                        lc = np_.tile([1, 512], f32, tag="lc")
                        nc.vector.tensor_copy(lc[:],
                                              pos[lh][HD:HD + 1, :])
                        rc = np_.tile([1, 512], f32, tag="rc", bufs=8,
                                      name=f"rc{t}{lh}")
                        nc.vector.reciprocal_approx_fast(rc[:], lc[:])
                        norm_tiles.append((t, lh, ol, rc))
                    if b == 1:
                        # queue b1 t's normalization + its out_proj tile
                        for lh in range(2):
                            _, _, ol, rc = norm_tiles[2 * t + lh]
                            cfill.append((emit_norm, (1, t, lh, ol, rc)))
                        if t == 3:
                            push_outproj(3)  # reserved: post-loop filler
                        push_outproj(4 + t)
                if b == 0:
                    drain_filler(len(filler))
                    # b0's norm + out_proj units drain during b1's chunks
                    for t, lh, ol, rc in norm_tiles:
                        cfill.append((emit_norm, (0, t, lh, ol, rc)))
                    for n in range(3):
                        push_outproj(n)
                    ctxQ.close()  # free psQ, open psF in its place
                    psF_box[0] = ctx.enter_context(
                        tc.tile_pool(name="psF", bufs=2, space="PSUM"))
                else:
                    in_loop[0] = False
                    # warmers over the rc(t3) wait: po-ring slots free up
                    # early here, so these run while DVE finishes the
                    # reciprocal chain and keep the clock gate at 8/8
                    for w_i in range(6):
                        wt = psO.tile([HD + 1, 512], f32, tag="po",
                                      name=f"wu{w_i}")
                        nc.tensor.matmul(wt[0:64, 0:128], ident[:, 0:64],
                                         ident[:], start=True, stop=True)
                    drain_cfill(len(cfill))

            ctx.close()

    nc.compile()
    return nc


def _classify_mask(mask):
    m = np.asarray(mask).reshape(S, S) != 0
    if m.all():
        return "none", None
    if np.array_equal(m, np.tril(np.ones((S, S), bool))):
        return "causal", None
    return "general", m.T.astype(np.float32)


def _ensure_ntff_hook():
    """Register antenv.axon_hooks with a ctypes NTFF profile hook if the
    container image lacks it (mirrors trn_agent_boot's registration)."""
    import types
    try:
        from antenv.axon_hooks import get_axon_ntff_profile_hook  # noqa: F401
        return
    except ImportError:
        pass
    import contextlib
    import ctypes

    hook = None
    so_path = "/opt/axon/libaxon_pjrt.so"
    try:
        lib = ctypes.CDLL(so_path)
        if hasattr(lib, "axon_start_nrt_profile"):
            lib.axon_start_nrt_profile.argtypes = [
                ctypes.POINTER(ctypes.c_int64), ctypes.c_size_t]
            lib.axon_start_nrt_profile.restype = ctypes.c_int64
            lib.axon_stop_nrt_profile.argtypes = [ctypes.c_char_p]
            lib.axon_stop_nrt_profile.restype = ctypes.c_int64

            @contextlib.contextmanager
            def _hook(output_dir, device_ids):
                import jax
                jax.devices()
                if device_ids:
                    ids = (ctypes.c_int64 * len(device_ids))(*device_ids)
                    rc = lib.axon_start_nrt_profile(ids, len(device_ids))
                else:
                    rc = lib.axon_start_nrt_profile(None, 0)
                if rc != 0:
                    raise RuntimeError(f"axon_start_nrt_profile rc={rc}")
                try:
                    yield
                finally:
                    n = lib.axon_stop_nrt_profile(str(output_dir).encode())
                    print(f"profile: {n} file(s) written to {output_dir}",
                          flush=True)

            hook = _hook
    except OSError:
        pass

    mod = types.ModuleType("antenv.axon_hooks")
    _h = [hook]
    mod.get_axon_ntff_profile_hook = lambda: _h[0]

    def _set(h):
        _h[0] = h

    mod.set_axon_ntff_profile_hook = _set
    sys.modules["antenv.axon_hooks"] = mod
    try:
        import antenv
        antenv.axon_hooks = mod
    except ImportError:
        pass


def kernel(key, query, value, mask, W_qkv, W_out, b_out):
    from concourse.bass_utils import run_bass_kernel_spmd
    import os

    mask_mode, maskT = _classify_mask(mask)
    if mask_mode not in _cache:
        _cache[mask_mode] = _build(mask_mode)
    nc = _cache[mask_mode]

    x = np.ascontiguousarray(
        np.asarray(query, np.float32).reshape(BS, D))
    xT_bf = np.ascontiguousarray(x.T).astype(BF16)
    # blocked layout [p, n, ec, s]: 8KB-contiguous per (partition, n-tile)
    xh = np.ascontiguousarray(
        xT_bf.reshape(EC, 128, NT, 512).transpose(1, 2, 0, 3)
    ).reshape(128, NT * EC * 512)
    W_qkv = np.asarray(W_qkv, np.float32)
    W_out = np.asarray(W_out, np.float32)

    consts = np.zeros((128, 256), BF16)
    consts[:, 0:128] = np.eye(128, dtype=BF16)
    consts[:, 128:256] = np.tril(np.ones((128, 128), np.float32)
                                 ).T.astype(BF16)

    in_maps = []
    for c in range(NCORES):
        sl = slice(SEC * c, SEC * (c + 1))
        wq = W_qkv[sl, :].T * np.float32(HD ** -0.5)
        wk = W_qkv[D + SEC * c:D + SEC * (c + 1), :].T
        wv = W_qkv[2 * D + SEC * c:2 * D + SEC * (c + 1), :].T
        wcat = np.concatenate([wq, wk, wv], axis=1,
                              dtype=np.float32).astype(BF16)
        m = {
            "xh": xh,
            "consts": consts,
            # blocked [p, ec, c] so per-partition runs are 6KB
            "wqh": np.ascontiguousarray(
                wcat.reshape(EC, 128, 3 * SEC).transpose(1, 0, 2)
            ).reshape(128, EC * 3 * SEC),
            "woT": np.ascontiguousarray(W_out[:, sl].T).astype(BF16),
        }
        if mask_mode == "general":
            m["maskT"] = maskT.astype(BF16)
        in_maps.append(m)

    trace = bool(int(os.environ.get("KERNEL_TRACE", "0")))
    if trace:
        _ensure_ntff_hook()
        try:
            res = run_bass_kernel_spmd(nc, in_maps,
                                       core_ids=list(range(NCORES)),
                                       trace=True)
        except Exception as e:
            print(f"traced run failed ({e!r}); retrying untraced",
                  flush=True)
            res = run_bass_kernel_spmd(nc, in_maps,
                                       core_ids=list(range(NCORES)))
        print(f"HW exec time: {res.exec_time_ns} ns", flush=True)
        kernel.last_exec_ns = res.exec_time_ns
        kernel.last_results = res
    else:
        res = run_bass_kernel_spmd(nc, in_maps, core_ids=list(range(NCORES)))

    acc = res.results[0]["out_pT"].astype(np.float32)
    for c in range(1, NCORES):
        acc = acc + res.results[c]["out_pT"].astype(np.float32)
    out = acc.T.reshape(B, S, D) + np.asarray(b_out, np.float32)
    return out.astype(np.float32)


# revision 29
# speedup vs baseline: 1.1594x; 1.1594x over previous
"""Multi-head attention forward, sharded head-parallel across 8 NeuronCores.

Per core c (heads 2c, 2c+1):
  qT/kT/vT = (x @ W{q,k,v}_c.T).T        computed as W.T-tiled matmuls vs xT
  scoresT  = kT_chunk.T @ qT             [k-pos partitions, q-pos free]
  probsT   = exp(scoresT), causal handled by suffix windows + DVE tri-mul
  av+rowsum: out.T = [v | 1].T @ probsT  (ones column yields softmax denom)
  normalize by PE-broadcast reciprocal, then out_projT partial (bf16)
Host: sum the 8 partial [1024, 4096] outputs, transpose, add bias.
"""
import sys
from contextlib import ExitStack

sys.path.insert(0, "/opt/trn_rl_repo")

import ml_dtypes
import numpy as np

BF16 = ml_dtypes.bfloat16

B, S, D = 2, 2048, 1024
H, HD = 16, 64
NCORES = 8
SEC = 128           # output dims per core per section (2 heads * 64)
BS = B * S          # 4096
NT = BS // 512      # 8 seq tiles of 512
EC = D // 128       # 8 embed chunks
QT = S // 512       # 4 q-tiles per (b,h)
KC = S // 128       # 16 k-chunks per (b,h)

_cache = {}


def _build(mask_mode):
    import concourse.bass as bass
    import concourse.tile as tile
    from concourse import bacc, mybir

    f32 = mybir.dt.float32
    bf16 = mybir.dt.bfloat16
    Exp = mybir.ActivationFunctionType.Exp

    nc = bacc.Bacc("TRN2", target_bir_lowering=False, debug=False,
                   num_devices=NCORES)

    # xh: host-blocked x layout [128, n, ec, 512] so each per-n DMA reads
    # 8KB-contiguous runs per partition
    xh = nc.dram_tensor("xh", [128, NT * EC * 512], bf16,
                        kind="ExternalInput")
    # wqh: host-blocked weights [128, ec, 3*SEC] -> 6KB runs per partition
    wqh = nc.dram_tensor("wqh", [128, EC * 3 * SEC], bf16,
                         kind="ExternalInput")
    woT = nc.dram_tensor("woT", [SEC, D], bf16, kind="ExternalInput")
    # consts: [:, 0:128] = eye(128); [:, 128:256] = lower-tri (p <= c)
    consts = nc.dram_tensor("consts", [128, 256], bf16, kind="ExternalInput")
    if mask_mode == "general":
        maskT = nc.dram_tensor("maskT", [S, S], bf16, kind="ExternalInput")
    out_pT = nc.dram_tensor("out_pT", [D, BS], bf16, kind="ExternalOutput")

    with tile.TileContext(nc) as tc:
        with (
            nc.allow_low_precision(reason="bf16 transpose/accum passthrough"),
            tc.tile_pool(name="singles", bufs=1) as singles,
            tc.tile_pool(name="qkv", bufs=1) as qkv,
            tc.tile_pool(name="xp", bufs=4) as xp,
            tc.tile_pool(name="v1p", bufs=1) as v1p,
            tc.tile_pool(name="pp", bufs=4) as pp,
            tc.tile_pool(name="np_", bufs=2) as np_,
            tc.tile_pool(name="fo", bufs=2) as fo,
        ):
            # ---- ident + qkv weights head the sync (hardware DMA) queue;
            # the gpsimd software-DGE queue is slow to start, so only the
            # late-needed woT/tri ride it ----
            ident = singles.tile([128, 128], bf16)
            nc.sync.dma_start(out=ident[:], in_=consts[:, 0:128])
            w_sb = singles.tile([128, EC, 3 * SEC], bf16)
            nc.sync.dma_start(
                out=w_sb[:],
                in_=wqh.rearrange("p (ec c) -> p ec c", ec=EC)[:])
            woT_sb = singles.tile([128, D], bf16)
            nc.gpsimd.dma_start(out=woT_sb[:], in_=woT[:])
            tri = singles.tile([128, 128], bf16)
            nc.gpsimd.dma_start(out=tri[:], in_=consts[:, 128:256])
            ones1 = singles.tile([1, 64], f32)
            nc.vector.memset(ones1[:], 1.0)

            # ---- x load: per-n tiles, n-major; 8KB runs per partition ----
            xhr = xh.rearrange("p (n ec s) -> p n ec s", n=NT, ec=EC)
            xns = []
            for n in range(NT):
                xn = qkv.tile([128, EC, 512], bf16, name=f"xn{n}")
                nc.sync.dma_start(out=xn[:], in_=xhr[:, n, :, :])
                xns.append(xn)
            qT = qkv.tile([128, BS], bf16)
            kT = qkv.tile([128, BS], bf16)
            vT = qkv.tile([128, BS], bf16)
            ocat = qkv.tile([128, BS], bf16)

            # v1[b]: [128 kpos, chunk, head, 64 v-dims + ones col]
            v1s = []
            for b in range(B):
                v1 = v1p.tile([128, KC, 2, HD + 1], bf16, name=f"v1{b}")
                nc.vector.memset(v1[:, :, :, HD], 1.0)
                v1s.append(v1)

            ctx = ExitStack()
            # scores/pos pools open up front so attention never waits on a
            # pool handoff; psQ is the slim QKV accumulator ring
            psS = ctx.enter_context(
                tc.tile_pool(name="psS", bufs=2, space="PSUM"))
            psO = ctx.enter_context(
                tc.tile_pool(name="psO", bufs=2, space="PSUM"))
            ctxQ = ExitStack()
            psQ = ctxQ.enter_context(
                tc.tile_pool(name="psQ", bufs=2, space="PSUM"))
            psF_box = [None]

            # PE warm-up: ~4us of dummy matmuls on ident while x loads, so
            # the HAM clock gate is already 8/8 when stage A starts
            warm = psS.tile([128, 128], f32, tag="ps", name="warm")
            for _ in range(75):
                nc.tensor.matmul(warm[:], ident[:], ident[:],
                                 start=True, stop=True)

            # transpose one [128 vdims, 128 seq] chunk -> v1[b][:, i, :, 0:64]
            def emit_transpose(b, i):
                base = S * b
                pt = psS.tile([128, 128], bf16, tag="ps", name="pt")
                nc.tensor.transpose(
                    pt[:], vT[:, base + 128 * i:base + 128 * (i + 1)],
                    ident[:])
                ptv = pt.rearrange("p (lh c) -> p lh c", lh=2)
                nc.vector.tensor_copy(v1s[b][:, i, :, 0:HD], ptv[:])

            # one q/k/v projection slab: 8 accumulating MMs + one cast,
            # emitted in two halves so filler stays fine-grained
            pa_box = {}

            def emit_proj_half(n, kind, dstT, half, scalar_cast):
                if half == 0:
                    pa_box[(n, kind)] = psQ.tile([128, 512], f32, tag="pa",
                                                 name=f"pa{n}_{kind}")
                pa = pa_box[(n, kind)]
                for ec in range(4 * half, 4 * half + 4):
                    nc.tensor.matmul(
                        pa[:], w_sb[:, ec, 128 * kind:128 * (kind + 1)],
                        xns[n][:, ec, :], start=(ec == 0), stop=(ec == EC - 1))
                if half == 1:
                    sl = slice(512 * n, 512 * (n + 1))
                    if scalar_cast:
                        nc.scalar.copy(dstT[:, sl], pa[:])
                    else:
                        nc.vector.tensor_copy(dstT[:, sl], pa[:])
                    del pa_box[(n, kind)]

            def emit_proj(n, kind, dstT, scalar_cast):
                emit_proj_half(n, kind, dstT, 0, scalar_cast)
                emit_proj_half(n, kind, dstT, 1, scalar_cast)

            # ---- stage A part 1: n0-3 (batch 0) + its transposes ----
            for n in range(4):
                for kind, dstT in ((0, qT), (1, kT), (2, vT)):
                    emit_proj(n, kind, dstT, scalar_cast=True)
                for i in range(4 * (n - 1) if n else 0, 4 * n):
                    emit_transpose(0, i)
            for i in range(12, 16):
                emit_transpose(0, i)

            # filler thunks: QKV n4-7 + b1 transposes, fed into b0's
            # attention stream to fill PE gaps (in-order PE queue)
            filler = []
            for n in range(4, NT):
                for kind, dstT in ((0, qT), (1, kT), (2, vT)):
                    filler.append((emit_proj_half, (n, kind, dstT, 0, False)))
                    filler.append((emit_proj_half, (n, kind, dstT, 1, False)))
                for i in range(4 * (n - 4), 4 * (n - 3)):
                    filler.append((emit_transpose, (1, i)))
            fill_pos = [0]

            def drain_filler(k):
                while fill_pos[0] < len(filler) and k > 0:
                    fn, args = filler[fill_pos[0]]
                    fn(*args)
                    fill_pos[0] += 1
                    k -= 1

            def emit_av(v1, pos, pend, njc):
                ji, j, off, pr = pend
                prv = pr.rearrange("p (lh q) -> p lh q", lh=2)
                for lh in range(2):
                    nc.tensor.matmul(pos[lh][:, off:512],
                                     v1[:, j, lh, :],
                                     prv[:, lh, off:512],
                                     start=(ji == 0), stop=(ji == njc - 1))

            def emit_outproj(n, use_scalar):
                ssl = slice(512 * n, 512 * (n + 1))
                ft = fo.tile([128, EC, 512], bf16, tag="ft")
                for oc in range(EC):
                    osl = slice(128 * oc, 128 * (oc + 1))
                    pf = psF_box[0].tile([128, 512], f32, tag="pf")
                    nc.tensor.matmul(pf[:], woT_sb[:, osl],
                                     ocat[:, ssl],
                                     start=True, stop=True)
                    if use_scalar:
                        nc.scalar.copy(ft[:, oc, :], pf[:])
                    else:
                        nc.vector.tensor_copy(ft[:, oc, :], pf[:])
                out_r = out_pT.rearrange("(oc p) s -> p oc s", p=128)
                nc.sync.dma_start(out=out_r[:, :, ssl], in_=ft[:])

            def emit_norm(b, t, lh, ol, rc):
                base = S * b
                hsl = slice(64 * lh, 64 * (lh + 1))
                qsl = slice(base + 512 * t, base + 512 * (t + 1))
                pb = psO.tile([HD + 1, 512], f32, tag="po", name="pb")
                nc.tensor.matmul(pb[0:HD, :], ones1[:], rc[:],
                                 start=True, stop=True)
                bs_ = np_.tile([64, 512], bf16, tag="bs")
                nc.vector.tensor_copy(bs_[:], pb[0:HD, :])
                nc.gpsimd.tensor_mul(ocat[hsl, qsl], ol[:], bs_[:])

            # b1-era fill queue: norm + out_proj units, drained a few per
            # chunk so the PE stream stays dense through the diag-chunk eras
            cfill = []
            cpos = [0]
            in_loop = [True]
            ft_box = {}
            out_r = out_pT.rearrange("(oc p) s -> p oc s", p=128)

            def emit_oc(n, oc):
                if oc == 0:
                    ft_box[n] = fo.tile([128, EC, 512], bf16, tag="ft",
                                        name=f"ft{n}")
                ft = ft_box[n]
                ssl = slice(512 * n, 512 * (n + 1))
                pf = psF_box[0].tile([128, 512], f32, tag="pf")
                nc.tensor.matmul(pf[:], woT_sb[:, 128 * oc:128 * (oc + 1)],
                                 ocat[:, ssl], start=True, stop=True)
                if oc % 2 == 1:
                    nc.scalar.copy(ft[:, oc, :], pf[:])
                else:
                    nc.vector.tensor_copy(ft[:, oc, :], pf[:])
                if oc % 2 == 1:
                    ssl = slice(512 * n, 512 * (n + 1))
                    nc.sync.dma_start(out=out_r[:, oc - 1:oc + 1, ssl],
                                      in_=ft[:, oc - 1:oc + 1, :])
                    if oc == EC - 1:
                        ft_box.pop(n)

            def push_outproj(n):
                for oc in range(EC):
                    cfill.append((emit_oc, (n, oc)))

            def drain_cfill(k):
                while cpos[0] < len(cfill) and k > 0:
                    fn, args = cfill[cpos[0]]
                    fn(*args)
                    cpos[0] += 1
                    k -= 1

            # ---- stage B (+ C interleaved into b1) ----
            for b in range(B):
                base = S * b
                v1 = v1s[b]
                norm_tiles = []
                for t in range(QT):
                    njc = 4 * t + 4 if mask_mode == "causal" else KC
                    pos = [psO.tile([HD + 1, 512], f32, tag="po",
                                    name=f"po{_lh}")
                           for _lh in range(2)]
                    pend = None
                    for ji, j in enumerate(range(njc)):
                        diag = mask_mode == "causal" and j >= 4 * t
                        off = 128 * (j - 4 * t) if diag else 0
                        ksl = slice(base + 128 * j, base + 128 * (j + 1))
                        ps = psS.tile([128, 1024], f32, tag="ps")
                        for lh in range(2):
                            hsl = slice(64 * lh, 64 * (lh + 1))
                            qsl = slice(base + 512 * t + off,
                                        base + 512 * (t + 1))
                            nc.tensor.matmul(
                                ps[:, 512 * lh + off:512 * (lh + 1)],
                                kT[hsl, ksl], qT[hsl, qsl],
                                start=True, stop=True)
                        pr = pp.tile([128, 1024], bf16, tag="pr")
                        psv = ps.rearrange("p (lh q) -> p lh q", lh=2)
                        prv = pr.rearrange("p (lh q) -> p lh q", lh=2)
                        nc.scalar.activation(prv[:, :, off:512],
                                             psv[:, :, off:512], Exp)
                        if diag:
                            for lh in range(2):
                                sub = pr[:, 512 * lh + off:
                                         512 * lh + off + 128]
                                nc.vector.tensor_mul(sub, sub, tri[:])
                        elif mask_mode == "general":
                            msk = xp.tile([128, 512], bf16, tag="msk")
                            nc.sync.dma_start(
                                out=msk[:],
                                in_=maskT[128 * j:128 * (j + 1),
                                          512 * t:512 * (t + 1)])
                            for lh in range(2):
                                sub = pr[:, 512 * lh:512 * (lh + 1)]
                                nc.vector.tensor_mul(sub, sub, msk[:])
                        if pend is not None:
                            emit_av(v1, pos, pend, njc)
                        pend = (ji, j, off, pr)
                        if b == 0:
                            drain_filler(1)
                        else:
                            drain_cfill(2 if t < 2 else 3)
                    emit_av(v1, pos, pend, njc)
                    for lh in range(2):
                        ol = np_.tile([HD, 512], f32, tag="ol", bufs=8,
                                      name=f"ol{t}{lh}")
                        nc.vector.tensor_copy(ol[:], pos[lh][0:HD, :])
                        lc = np_.tile([1, 512], f32, tag="lc")
                        nc.vector.tensor_copy(lc[:],
                                              pos[lh][HD:HD + 1, :])
                        rc = np_.tile([1, 512], f32, tag="rc", bufs=8,
                                      name=f"rc{t}{lh}")
                        nc.vector.reciprocal_approx_fast(rc[:], lc[:])
                        norm_tiles.append((t, lh, ol, rc))
                    if b == 1:
                        # queue b1 t's normalization + its out_proj tile
                        for lh in range(2):
                            _, _, ol, rc = norm_tiles[2 * t + lh]
                            cfill.append((emit_norm, (1, t, lh, ol, rc)))
                        push_outproj(4 + t)
                if b == 0:
                    drain_filler(len(filler))
                    # b0's norm + out_proj units drain during b1's chunks
                    for t, lh, ol, rc in norm_tiles:
                        cfill.append((emit_norm, (0, t, lh, ol, rc)))
                    for n in range(4):
                        push_outproj(n)
                    ctxQ.close()  # free psQ, open psF in its place
                    psF_box[0] = ctx.enter_context(
                        tc.tile_pool(name="psF", bufs=2, space="PSUM"))
                else:
                    in_loop[0] = False
                    # warmers over the rc(t3) wait: po-ring slots free up
                    # early here, so these run while DVE finishes the
                    # reciprocal chain and keep the clock gate at 8/8
                    for w_i in range(6):
                        wt = psO.tile([HD + 1, 512], f32, tag="po",
                                      name=f"wu{w_i}")
                        nc.tensor.matmul(wt[0:64, 0:128], ident[:, 0:64],
                                         ident[:], start=True, stop=True)
                    drain_cfill(len(cfill))

            ctx.close()

    nc.compile()
    return nc


def _classify_mask(mask):
    m = np.asarray(mask).reshape(S, S) != 0
    if m.all():
        return "none", None
    if np.array_equal(m, np.tril(np.ones((S, S), bool))):
        return "causal", None
    return "general", m.T.astype(np.float32)


def _ensure_ntff_hook():
    """Register antenv.axon_hooks with a ctypes NTFF profile hook if the
    container image lacks it (mirrors trn_agent_boot's registration)."""
    import types
    try:
        from antenv.axon_hooks import get_axon_ntff_profile_hook  # noqa: F401
        return
    except ImportError:
        pass
    import contextlib
    import ctypes

    hook = None
    so_path = "/opt/axon/libaxon_pjrt.so"
    try:
        lib = ctypes.CDLL(so_path)
        if hasattr(lib, "axon_start_nrt_profile"):
            lib.axon_start_nrt_profile.argtypes = [
                ctypes.POINTER(ctypes.c_int64), ctypes.c_size_t]
            lib.axon_start_nrt_profile.restype = ctypes.c_int64
            lib.axon_stop_nrt_profile.argtypes = [ctypes.c_char_p]
            lib.axon_stop_nrt_profile.restype = ctypes.c_int64

            @contextlib.contextmanager
            def _hook(output_dir, device_ids):
                import jax
                jax.devices()
                if device_ids:
                    ids = (ctypes.c_int64 * len(device_ids))(*device_ids)
                    rc = lib.axon_start_nrt_profile(ids, len(device_ids))
                else:
                    rc = lib.axon_start_nrt_profile(None, 0)
                if rc != 0:
                    raise RuntimeError(f"axon_start_nrt_profile rc={rc}")
                try:
                    yield
                finally:
                    n = lib.axon_stop_nrt_profile(str(output_dir).encode())
                    print(f"profile: {n} file(s) written to {output_dir}",
                          flush=True)

            hook = _hook
    except OSError:
        pass

    mod = types.ModuleType("antenv.axon_hooks")
    _h = [hook]
    mod.get_axon_ntff_profile_hook = lambda: _h[0]

    def _set(h):
        _h[0] = h

    mod.set_axon_ntff_profile_hook = _set
    sys.modules["antenv.axon_hooks"] = mod
    try:
        import antenv
        antenv.axon_hooks = mod
    except ImportError:
        pass


def kernel(key, query, value, mask, W_qkv, W_out, b_out):
    from concourse.bass_utils import run_bass_kernel_spmd
    import os

    mask_mode, maskT = _classify_mask(mask)
    if mask_mode not in _cache:
        _cache[mask_mode] = _build(mask_mode)
    nc = _cache[mask_mode]

    x = np.ascontiguousarray(
        np.asarray(query, np.float32).reshape(BS, D))
    xT_bf = np.ascontiguousarray(x.T).astype(BF16)
    # blocked layout [p, n, ec, s]: 8KB-contiguous per (partition, n-tile)
    xh = np.ascontiguousarray(
        xT_bf.reshape(EC, 128, NT, 512).transpose(1, 2, 0, 3)
    ).reshape(128, NT * EC * 512)
    W_qkv = np.asarray(W_qkv, np.float32)
    W_out = np.asarray(W_out, np.float32)

    consts = np.zeros((128, 256), BF16)
    consts[:, 0:128] = np.eye(128, dtype=BF16)
    consts[:, 128:256] = np.tril(np.ones((128, 128), np.float32)
                                 ).T.astype(BF16)

    in_maps = []
    for c in range(NCORES):
        sl = slice(SEC * c, SEC * (c + 1))
        wq = W_qkv[sl, :].T * np.float32(HD ** -0.5)
        wk = W_qkv[D + SEC * c:D + SEC * (c + 1), :].T
        wv = W_qkv[2 * D + SEC * c:2 * D + SEC * (c + 1), :].T
        wcat = np.concatenate([wq, wk, wv], axis=1,
                              dtype=np.float32).astype(BF16)
        m = {
            "xh": xh,
            "consts": consts,
            # blocked [p, ec, c] so per-partition runs are 6KB
            "wqh": np.ascontiguousarray(
                wcat.reshape(EC, 128, 3 * SEC).transpose(1, 0, 2)
            ).reshape(128, EC * 3 * SEC),
            "woT": np.ascontiguousarray(W_out[:, sl].T).astype(BF16),
        }
        if mask_mode == "general":
            m["maskT"] = maskT.astype(BF16)
        in_maps.append(m)

    trace = bool(int(os.environ.get("KERNEL_TRACE", "0")))
    if trace:
        _ensure_ntff_hook()
        try:
            res = run_bass_kernel_spmd(nc, in_maps,
                                       core_ids=list(range(NCORES)),
                                       trace=True)
        except Exception as e:
            print(f"traced run failed ({e!r}); retrying untraced",
                  flush=True)
            res = run_bass_kernel_spmd(nc, in_maps,
                                       core_ids=list(range(NCORES)))
        print(f"HW exec time: {res.exec_time_ns} ns", flush=True)
        kernel.last_exec_ns = res.exec_time_ns
        kernel.last_results = res
    else:
        res = run_bass_kernel_spmd(nc, in_maps, core_ids=list(range(NCORES)))

    acc = res.results[0]["out_pT"].astype(np.float32)
    for c in range(1, NCORES):
        acc = acc + res.results[c]["out_pT"].astype(np.float32)
    out = acc.T.reshape(B, S, D) + np.asarray(b_out, np.float32)
    return out.astype(np.float32)


# revision 31
# speedup vs baseline: 1.1598x; 1.0004x over previous
"""Multi-head attention forward, sharded head-parallel across 8 NeuronCores.

Per core c (heads 2c, 2c+1):
  qT/kT/vT = (x @ W{q,k,v}_c.T).T        computed as W.T-tiled matmuls vs xT
  scoresT  = kT_chunk.T @ qT             [k-pos partitions, q-pos free],
             both heads concurrent on PE row-groups 0-63 / 64-127
  probsT   = exp(scoresT); causal mask via suffix windows (scores/exp/AV
             restricted to the valid q-range) + one DVE tri-mul per diag
  av+rowsum: out.T = [v | 1].T @ probsT  (ones column yields softmax denom)
  normalize by PE-broadcast reciprocal, then out_projT partial (bf16)
Host: sum the 8 partial [1024, 4096] bf16 outputs, transpose, add bias.

Scheduling: inputs arrive via host-blocked layouts (8KB DMA runs) on the
hardware sync queue; dummy ident-matmuls warm the PE clock gate during the
load; QKV for batch 1 and the v-transposes are fed as filler into batch 0's
attention stream; batch 0's normalization + out-projection fill batch 1's
stream (PE queues are in-order, so emission order is the schedule).
"""
import sys
from contextlib import ExitStack

sys.path.insert(0, "/opt/trn_rl_repo")

import ml_dtypes
import numpy as np

BF16 = ml_dtypes.bfloat16

B, S, D = 2, 2048, 1024
H, HD = 16, 64
NCORES = 8
SEC = 128           # output dims per core per section (2 heads * 64)
BS = B * S          # 4096
NT = BS // 512      # 8 seq tiles of 512
EC = D // 128       # 8 embed chunks
QT = S // 512       # 4 q-tiles per (b,h)
KC = S // 128       # 16 k-chunks per (b,h)

_cache = {}


def _build(mask_mode):
    import concourse.bass as bass
    import concourse.tile as tile
    from concourse import bacc, mybir

    f32 = mybir.dt.float32
    bf16 = mybir.dt.bfloat16
    Exp = mybir.ActivationFunctionType.Exp

    nc = bacc.Bacc("TRN2", target_bir_lowering=False, debug=False,
                   num_devices=NCORES)

    # xh: host-blocked x layout [128, n, ec, 512] so each per-n DMA reads
    # 8KB-contiguous runs per partition
    xh = nc.dram_tensor("xh", [128, NT * EC * 512], bf16,
                        kind="ExternalInput")
    # wqh: host-blocked weights [128, ec, 3*SEC] -> 6KB runs per partition
    wqh = nc.dram_tensor("wqh", [128, EC * 3 * SEC], bf16,
                         kind="ExternalInput")
    woT = nc.dram_tensor("woT", [SEC, D], bf16, kind="ExternalInput")
    # consts: [:, 0:128] = eye(128); [:, 128:256] = lower-tri (p <= c)
    consts = nc.dram_tensor("consts", [128, 256], bf16, kind="ExternalInput")
    if mask_mode == "general":
        maskT = nc.dram_tensor("maskT", [S, S], bf16, kind="ExternalInput")
    out_pT = nc.dram_tensor("out_pT", [D, BS], bf16, kind="ExternalOutput")

    with tile.TileContext(nc) as tc:
        with (
            nc.allow_low_precision(reason="bf16 transpose/accum passthrough"),
            tc.tile_pool(name="singles", bufs=1) as singles,
            tc.tile_pool(name="qkv", bufs=1) as qkv,
            tc.tile_pool(name="xp", bufs=4) as xp,
            tc.tile_pool(name="v1p", bufs=1) as v1p,
            tc.tile_pool(name="pp", bufs=4) as pp,
            tc.tile_pool(name="np_", bufs=2) as np_,
            tc.tile_pool(name="fo", bufs=2) as fo,
        ):
            # ---- ident + qkv weights head the sync (hardware DMA) queue;
            # the gpsimd software-DGE queue is slow to start, so only the
            # late-needed woT/tri ride it ----
            ident = singles.tile([128, 128], bf16)
            nc.sync.dma_start(out=ident[:], in_=consts[:, 0:128])
            w_sb = singles.tile([128, EC, 3 * SEC], bf16)
            nc.sync.dma_start(
                out=w_sb[:],
                in_=wqh.rearrange("p (ec c) -> p ec c", ec=EC)[:])
            woT_sb = singles.tile([128, D], bf16)
            nc.gpsimd.dma_start(out=woT_sb[:], in_=woT[:])
            tri = singles.tile([128, 128], bf16)
            nc.gpsimd.dma_start(out=tri[:], in_=consts[:, 128:256])
            ones1 = singles.tile([1, 64], f32)
            nc.vector.memset(ones1[:], 1.0)

            # ---- x load: per-n tiles, n-major; 8KB runs per partition ----
            xhr = xh.rearrange("p (n ec s) -> p n ec s", n=NT, ec=EC)
            xns = []
            for n in range(NT):
                xn = qkv.tile([128, EC, 512], bf16, name=f"xn{n}")
                nc.sync.dma_start(out=xn[:], in_=xhr[:, n, :, :])
                xns.append(xn)
            qT = qkv.tile([128, BS], bf16)
            kT = qkv.tile([128, BS], bf16)
            vT = qkv.tile([128, BS], bf16)
            ocat = qkv.tile([128, BS], bf16)

            # v1[b]: [128 kpos, chunk, head, 64 v-dims + ones col]
            v1s = []
            for b in range(B):
                v1 = v1p.tile([128, KC, 2, HD + 1], bf16, name=f"v1{b}")
                nc.vector.memset(v1[:, :, :, HD], 1.0)
                v1s.append(v1)

            ctx = ExitStack()
            # scores/pos pools open up front so attention never waits on a
            # pool handoff; psQ is the slim QKV accumulator ring
            psS = ctx.enter_context(
                tc.tile_pool(name="psS", bufs=2, space="PSUM"))
            psO = ctx.enter_context(
                tc.tile_pool(name="psO", bufs=2, space="PSUM"))
            ctxQ = ExitStack()
            psQ = ctxQ.enter_context(
                tc.tile_pool(name="psQ", bufs=2, space="PSUM"))
            psF_box = [None]

            # PE warm-up: ~4us of dummy matmuls on ident while x loads, so
            # the HAM clock gate is already 8/8 when stage A starts
            warm = psS.tile([128, 128], f32, tag="ps", name="warm")
            for _ in range(75):
                nc.tensor.matmul(warm[:], ident[:], ident[:],
                                 start=True, stop=True)

            # transpose one [128 vdims, 128 seq] chunk -> v1[b][:, i, :, 0:64]
            def emit_transpose(b, i):
                base = S * b
                pt = psS.tile([128, 128], bf16, tag="ps", name="pt")
                nc.tensor.transpose(
                    pt[:], vT[:, base + 128 * i:base + 128 * (i + 1)],
                    ident[:])
                ptv = pt.rearrange("p (lh c) -> p lh c", lh=2)
                nc.vector.tensor_copy(v1s[b][:, i, :, 0:HD], ptv[:])

            # one q/k/v projection slab: 8 accumulating MMs + one cast,
            # emitted in two halves so filler stays fine-grained
            pa_box = {}

            def emit_proj_half(n, kind, dstT, half, scalar_cast):
                if half == 0:
                    pa_box[(n, kind)] = psQ.tile([128, 512], f32, tag="pa",
                                                 name=f"pa{n}_{kind}")
                pa = pa_box[(n, kind)]
                for ec in range(4 * half, 4 * half + 4):
                    nc.tensor.matmul(
                        pa[:], w_sb[:, ec, 128 * kind:128 * (kind + 1)],
                        xns[n][:, ec, :], start=(ec == 0), stop=(ec == EC - 1))
                if half == 1:
                    sl = slice(512 * n, 512 * (n + 1))
                    if scalar_cast:
                        nc.scalar.copy(dstT[:, sl], pa[:])
                    else:
                        nc.vector.tensor_copy(dstT[:, sl], pa[:])
                    del pa_box[(n, kind)]

            def emit_proj(n, kind, dstT, scalar_cast):
                emit_proj_half(n, kind, dstT, 0, scalar_cast)
                emit_proj_half(n, kind, dstT, 1, scalar_cast)

            # ---- stage A part 1: n0-3 (batch 0) + its transposes ----
            for n in range(4):
                for kind, dstT in ((0, qT), (1, kT), (2, vT)):
                    emit_proj(n, kind, dstT, scalar_cast=True)
                for i in range(4 * (n - 1) if n else 0, 4 * n):
                    emit_transpose(0, i)
            for i in range(12, 16):
                emit_transpose(0, i)

            # filler thunks: QKV n4-7 + b1 transposes, fed into b0's
            # attention stream to fill PE gaps (in-order PE queue)
            filler = []
            for n in range(4, NT):
                for kind, dstT in ((0, qT), (1, kT), (2, vT)):
                    filler.append((emit_proj_half, (n, kind, dstT, 0, False)))
                    filler.append((emit_proj_half, (n, kind, dstT, 1, False)))
                for i in range(4 * (n - 4), 4 * (n - 3)):
                    filler.append((emit_transpose, (1, i)))
            fill_pos = [0]

            def drain_filler(k):
                while fill_pos[0] < len(filler) and k > 0:
                    fn, args = filler[fill_pos[0]]
                    fn(*args)
                    fill_pos[0] += 1
                    k -= 1

            def emit_av(v1, pos, pend, njc):
                ji, j, off, pr = pend
                prv = pr.rearrange("p (lh q) -> p lh q", lh=2)
                for lh in range(2):
                    nc.tensor.matmul(pos[lh][:, off:512],
                                     v1[:, j, lh, :],
                                     prv[:, lh, off:512],
                                     start=(ji == 0), stop=(ji == njc - 1))

            def emit_outproj(n, use_scalar):
                ssl = slice(512 * n, 512 * (n + 1))
                ft = fo.tile([128, EC, 512], bf16, tag="ft")
                for oc in range(EC):
                    osl = slice(128 * oc, 128 * (oc + 1))
                    pf = psF_box[0].tile([128, 512], f32, tag="pf")
                    nc.tensor.matmul(pf[:], woT_sb[:, osl],
                                     ocat[:, ssl],
                                     start=True, stop=True)
                    if use_scalar:
                        nc.scalar.copy(ft[:, oc, :], pf[:])
                    else:
                        nc.vector.tensor_copy(ft[:, oc, :], pf[:])
                out_r = out_pT.rearrange("(oc p) s -> p oc s", p=128)
                nc.sync.dma_start(out=out_r[:, :, ssl], in_=ft[:])

            def emit_norm(b, t, lh, ol, rc):
                base = S * b
                hsl = slice(64 * lh, 64 * (lh + 1))
                qsl = slice(base + 512 * t, base + 512 * (t + 1))
                pb = psO.tile([HD + 1, 512], f32, tag="po", name="pb")
                nc.tensor.matmul(pb[0:HD, :], ones1[:], rc[:],
                                 start=True, stop=True)
                bs_ = np_.tile([64, 512], bf16, tag="bs")
                nc.vector.tensor_copy(bs_[:], pb[0:HD, :])
                nc.gpsimd.tensor_mul(ocat[hsl, qsl], ol[:], bs_[:])

            # b1-era fill queue: norm + out_proj units, drained a few per
            # chunk so the PE stream stays dense through the diag-chunk eras
            cfill = []
            cpos = [0]
            in_loop = [True]
            ft_box = {}
            out_r = out_pT.rearrange("(oc p) s -> p oc s", p=128)

            def emit_oc(n, oc):
                if oc == 0:
                    ft_box[n] = fo.tile([128, EC, 512], bf16, tag="ft",
                                        name=f"ft{n}")
                ft = ft_box[n]
                ssl = slice(512 * n, 512 * (n + 1))
                pf = psF_box[0].tile([128, 512], f32, tag="pf")
                nc.tensor.matmul(pf[:], woT_sb[:, 128 * oc:128 * (oc + 1)],
                                 ocat[:, ssl], start=True, stop=True)
                if oc % 2 == 1:
                    nc.scalar.copy(ft[:, oc, :], pf[:])
                else:
                    nc.vector.tensor_copy(ft[:, oc, :], pf[:])
                if oc % 2 == 1:
                    ssl = slice(512 * n, 512 * (n + 1))
                    nc.sync.dma_start(out=out_r[:, oc - 1:oc + 1, ssl],
                                      in_=ft[:, oc - 1:oc + 1, :])
                    if oc == EC - 1:
                        ft_box.pop(n)

            def push_outproj(n):
                for oc in range(EC):
                    cfill.append((emit_oc, (n, oc)))

            def drain_cfill(k):
                while cpos[0] < len(cfill) and k > 0:
                    fn, args = cfill[cpos[0]]
                    fn(*args)
                    cpos[0] += 1
                    k -= 1

            # ---- stage B (+ C interleaved into b1) ----
            def emit_warmers(k, nm):
                for w_i in range(k):
                    wt = psO.tile([HD + 1, 512], f32, tag="po",
                                  name=f"{nm}{w_i}")
                    nc.tensor.matmul(wt[0:64, 0:128], ident[:, 0:64],
                                     ident[:], start=True, stop=True)

            for b in range(B):
                base = S * b
                v1 = v1s[b]
                norm_tiles = []
                if b == 1:
                    emit_warmers(4, "bw")
                for t in range(QT):
                    njc = 4 * t + 4 if mask_mode == "causal" else KC
                    pos = [psO.tile([HD + 1, 512], f32, tag="po",
                                    name=f"po{_lh}")
                           for _lh in range(2)]
                    pend = None
                    for ji, j in enumerate(range(njc)):
                        diag = mask_mode == "causal" and j >= 4 * t
                        off = 128 * (j - 4 * t) if diag else 0
                        ksl = slice(base + 128 * j, base + 128 * (j + 1))
                        ps = psS.tile([128, 1024], f32, tag="ps")
                        for lh in range(2):
                            hsl = slice(64 * lh, 64 * (lh + 1))
                            qsl = slice(base + 512 * t + off,
                                        base + 512 * (t + 1))
                            nc.tensor.matmul(
                                ps[:, 512 * lh + off:512 * (lh + 1)],
                                kT[hsl, ksl], qT[hsl, qsl],
                                start=True, stop=True)
                        pr = pp.tile([128, 1024], bf16, tag="pr")
                        psv = ps.rearrange("p (lh q) -> p lh q", lh=2)
                        prv = pr.rearrange("p (lh q) -> p lh q", lh=2)
                        nc.scalar.activation(prv[:, :, off:512],
                                             psv[:, :, off:512], Exp)
                        if diag:
                            for lh in range(2):
                                sub = pr[:, 512 * lh + off:
                                         512 * lh + off + 128]
                                nc.vector.tensor_mul(sub, sub, tri[:])
                        elif mask_mode == "general":
                            msk = xp.tile([128, 512], bf16, tag="msk")
                            nc.sync.dma_start(
                                out=msk[:],
                                in_=maskT[128 * j:128 * (j + 1),
                                          512 * t:512 * (t + 1)])
                            for lh in range(2):
                                sub = pr[:, 512 * lh:512 * (lh + 1)]
                                nc.vector.tensor_mul(sub, sub, msk[:])
                        if pend is not None:
                            emit_av(v1, pos, pend, njc)
                        pend = (ji, j, off, pr)
                        if b == 0:
                            drain_filler(1)
                        else:
                            drain_cfill(3)
                    emit_av(v1, pos, pend, njc)
                    for lh in range(2):
                        ol = np_.tile([HD, 512], f32, tag="ol", bufs=8,
                                      name=f"ol{t}{lh}")
                        nc.vector.tensor_copy(ol[:], pos[lh][0:HD, :])
                        lc = np_.tile([1, 512], f32, tag="lc")
                        nc.vector.tensor_copy(lc[:],
                                              pos[lh][HD:HD + 1, :])
                        rc = np_.tile([1, 512], f32, tag="rc", bufs=8,
                                      name=f"rc{t}{lh}")
                        nc.vector.reciprocal_approx_fast(rc[:], lc[:])
                        norm_tiles.append((t, lh, ol, rc))
                    if b == 1:
                        # queue b1 t's normalization + its out_proj tile
                        for lh in range(2):
                            _, _, ol, rc = norm_tiles[2 * t + lh]
                            cfill.append((emit_norm, (1, t, lh, ol, rc)))
                        push_outproj(4 + t)
                        if t < 3:
                            emit_warmers(2, f"tw{t}_")
                if b == 0:
                    drain_filler(len(filler))
                    # b0's norm + out_proj units drain during b1's chunks
                    for t, lh, ol, rc in norm_tiles:
                        cfill.append((emit_norm, (0, t, lh, ol, rc)))
                    for n in range(4):
                        push_outproj(n)
                    ctxQ.close()  # free psQ, open psF in its place
                    psF_box[0] = ctx.enter_context(
                        tc.tile_pool(name="psF", bufs=2, space="PSUM"))
                else:
                    in_loop[0] = False
                    # warmers over the rc(t3) wait: po-ring slots free up
                    # early here, so these run while DVE finishes the
                    # reciprocal chain and keep the clock gate at 8/8
                    for w_i in range(6):
                        wt = psO.tile([HD + 1, 512], f32, tag="po",
                                      name=f"wu{w_i}")
                        nc.tensor.matmul(wt[0:64, 0:128], ident[:, 0:64],
                                         ident[:], start=True, stop=True)
                    drain_cfill(len(cfill))

            ctx.close()

    nc.compile()
    return nc


def _classify_mask(mask):
    m = np.asarray(mask).reshape(S, S) != 0
    if m.all():
        return "none", None
    if np.array_equal(m, np.tril(np.ones((S, S), bool))):
        return "causal", None
    return "general", m.T.astype(np.float32)


def _ensure_ntff_hook():
    """Register antenv.axon_hooks with a ctypes NTFF profile hook if the
    container image lacks it (mirrors trn_agent_boot's registration)."""
    import types
    try:
        from antenv.axon_hooks import get_axon_ntff_profile_hook  # noqa: F401
        return
    except ImportError:
        pass
    import contextlib
    import ctypes

    hook = None
    so_path = "/opt/axon/libaxon_pjrt.so"
    try:
        lib = ctypes.CDLL(so_path)
        if hasattr(lib, "axon_start_nrt_profile"):
            lib.axon_start_nrt_profile.argtypes = [
                ctypes.POINTER(ctypes.c_int64), ctypes.c_size_t]
            lib.axon_start_nrt_profile.restype = ctypes.c_int64
            lib.axon_stop_nrt_profile.argtypes = [ctypes.c_char_p]
            lib.axon_stop_nrt_profile.restype = ctypes.c_int64

            @contextlib.contextmanager
            def _hook(output_dir, device_ids):
                import jax
                jax.devices()
                if device_ids:
                    ids = (ctypes.c_int64 * len(device_ids))(*device_ids)
                    rc = lib.axon_start_nrt_profile(ids, len(device_ids))
                else:
                    rc = lib.axon_start_nrt_profile(None, 0)
                if rc != 0:
                    raise RuntimeError(f"axon_start_nrt_profile rc={rc}")
                try:
                    yield
                finally:
                    n = lib.axon_stop_nrt_profile(str(output_dir).encode())
                    print(f"profile: {n} file(s) written to {output_dir}",
                          flush=True)

            hook = _hook
    except OSError:
        pass

    mod = types.ModuleType("antenv.axon_hooks")
    _h = [hook]
    mod.get_axon_ntff_profile_hook = lambda: _h[0]

    def _set(h):
        _h[0] = h

    mod.set_axon_ntff_profile_hook = _set
    sys.modules["antenv.axon_hooks"] = mod
    try:
        import antenv
        antenv.axon_hooks = mod
    except ImportError:
        pass


def kernel(key, query, value, mask, W_qkv, W_out, b_out):
    from concourse.bass_utils import run_bass_kernel_spmd
    import os

    mask_mode, maskT = _classify_mask(mask)
    if mask_mode not in _cache:
        _cache[mask_mode] = _build(mask_mode)
    nc = _cache[mask_mode]

    x = np.ascontiguousarray(
        np.asarray(query, np.float32).reshape(BS, D))
    xT_bf = np.ascontiguousarray(x.T).astype(BF16)
    # blocked layout [p, n, ec, s]: 8KB-contiguous per (partition, n-tile)
    xh = np.ascontiguousarray(
        xT_bf.reshape(EC, 128, NT, 512).transpose(1, 2, 0, 3)
    ).reshape(128, NT * EC * 512)
    W_qkv = np.asarray(W_qkv, np.float32)
    W_out = np.asarray(W_out, np.float32)

    consts = np.zeros((128, 256), BF16)
    consts[:, 0:128] = np.eye(128, dtype=BF16)
    consts[:, 128:256] = np.tril(np.ones((128, 128), np.float32)
                                 ).T.astype(BF16)

    in_maps = []
    for c in range(NCORES):
        sl = slice(SEC * c, SEC * (c + 1))
        wq = W_qkv[sl, :].T * np.float32(HD ** -0.5)
        wk = W_qkv[D + SEC * c:D + SEC * (c + 1), :].T
        wv = W_qkv[2 * D + SEC * c:2 * D + SEC * (c + 1), :].T
        wcat = np.concatenate([wq, wk, wv], axis=1,
                              dtype=np.float32).astype(BF16)
        m = {
            "xh": xh,
            "consts": consts,
            # blocked [p, ec, c] so per-partition runs are 6KB
            "wqh": np.ascontiguousarray(
                wcat.reshape(EC, 128, 3 * SEC).transpose(1, 0, 2)
            ).reshape(128, EC * 3 * SEC),
            "woT": np.ascontiguousarray(W_out[:, sl].T).astype(BF16),
        }
        if mask_mode == "general":
            m["maskT"] = maskT.astype(BF16)
        in_maps.append(m)

    trace = bool(int(os.environ.get("KERNEL_TRACE", "0")))
    if trace:
        _ensure_ntff_hook()
        try:
            res = run_bass_kernel_spmd(nc, in_maps,
                                       core_ids=list(range(NCORES)),
                                       trace=True)
        except Exception as e:
            print(f"traced run failed ({e!r}); retrying untraced",
                  flush=True)
            res = run_bass_kernel_spmd(nc, in_maps,
                                       core_ids=list(range(NCORES)))
        print(f"HW exec time: {res.exec_time_ns} ns", flush=True)
        kernel.last_exec_ns = res.exec_time_ns
        kernel.last_results = res
    else:
        res = run_bass_kernel_spmd(nc, in_maps, core_ids=list(range(NCORES)))

    acc = res.results[0]["out_pT"].astype(np.float32)
    for c in range(1, NCORES):
        acc = acc + res.results[c]["out_pT"].astype(np.float32)
    out = acc.T.reshape(B, S, D) + np.asarray(b_out, np.float32)
    return out.astype(np.float32)
